# revision 31
# baseline (speedup 1.0000x reference)
"""Multi-head attention (B=4, S=2048, D=1024, H=16) on 8 TRN2 NeuronCores. v3.

Sharding: core c = (batch c//2, seq-half c%2); K/V computed for the full
sequence on every core, Q/attention/outproj only for the core's half; host
rotates the sequence so the core's own half sits at columns [0, SH).
Everything on-chip is transposed (features on partitions): qT/kT [D,s],
scoresT [sk,sq], outT [d,sq].

Softmax: multiplicative mask => masked slots get weight exp(0)=1. ScalarE
exps the raw scores; ONE copy_predicated per 2 sk-chunks (FD=2048, predicate
= host-sent inverted mask, bf16 0/1) patches masked slots to 1.0. Head pairs
share one [128,1024] PSUM scores tile (two K=64 row-tiled matmuls). The
denominator comes free from the PV matmul via a ones column in packed v;
normalization is deferred (reciprocal + one-hot selector broadcast matmul).

v3 vs baseline: copy_predicated at FD=2048 (amortizes the DVE 1x overhead),
projection matmuls paired per stationary operand (the two N=512 column
blocks of each (mc,kc) share one LDWEIGHTS; one FD=1024 bias-activation per
pair), bf16 selector/reciprocal for the broadcast matmul.
"""

import os
import sys

import numpy as np

for _p in ("/opt/trn_rl_repo",):
    if _p not in sys.path and os.path.isdir(_p):
        sys.path.insert(0, _p)

import ml_dtypes

import concourse.bass as bass
from concourse import bacc
import concourse.mybir as mybir
import concourse.tile as tile
from concourse.bass_utils import run_bass_kernel_spmd

BF16 = mybir.dt.bfloat16
F32 = mybir.dt.float32
AF = mybir.ActivationFunctionType

B, S, D, H, DH = 4, 2048, 1024, 16, 64
NCORES = 8
SH = S // 2
P = 128
NB = 512
KC = D // P
SKC = S // P
SQB = SH // NB
NPAIR = H // 2
VW = DH + 1
VROW = H * VW
NROW = H * SQB

_bf16 = ml_dtypes.bfloat16


def _build_bass():
    nc = bacc.Bacc(num_devices=NCORES)

    qT_d = nc.declare_dram_parameter("qT", [D, S], BF16, isOutput=False)
    minvT_d = nc.declare_dram_parameter("minvT", [S, SH], mybir.dt.uint8, isOutput=False)
    mT_d = nc.declare_dram_parameter("mT", [S, SH], BF16, isOutput=False)
    vsum_d = nc.declare_dram_parameter("vsum", [P, KC], F32, isOutput=False)
    cnt_d = nc.declare_dram_parameter("cnt", [NROW, NB], F32, isOutput=False)
    wqT_d = nc.declare_dram_parameter("wqT", [D, D], BF16, isOutput=False)
    wkT_d = nc.declare_dram_parameter("wkT", [D, D], BF16, isOutput=False)
    wvT_d = nc.declare_dram_parameter("wvT", [D, D], BF16, isOutput=False)
    woT_d = nc.declare_dram_parameter("woT", [D, D], BF16, isOutput=False)
    bq_d = nc.declare_dram_parameter("bq", [D, 1], F32, isOutput=False)
    bk_d = nc.declare_dram_parameter("bk", [D, 1], F32, isOutput=False)
    bv_d = nc.declare_dram_parameter("bv", [P, D], F32, isOutput=False)
    bo_d = nc.declare_dram_parameter("bo", [D, 1], F32, isOutput=False)
    sel_d = nc.declare_dram_parameter("sel", [NROW, NPAIR * SQB * P], BF16, isOutput=False)
    out_d = nc.declare_dram_parameter("out", [D, SH], F32, isOutput=True)

    with tile.TileContext(nc) as tc:
        with (
            tc.tile_pool(name="persist", bufs=1) as persist,
            tc.tile_pool(name="psS", bufs=2, space="PSUM") as psS,
            tc.tile_pool(name="psV", bufs=1, space="PSUM") as psV,
        ):
            qT = persist.tile([P, KC * SH], BF16)
            kT = persist.tile([P, KC * S], BF16)
            vpk = persist.tile([P, SKC * VROW], BF16)
            outMT = persist.tile([P, KC * SH], BF16)
            sums_pk = persist.tile([NROW, NB], F32)
            recip_pk = persist.tile([NROW, NB], F32)
            recip_bf = persist.tile([NROW, NB], BF16)
            sel = persist.tile([NROW, NPAIR * SQB * P], BF16)
            nc.sync.dma_start(sel[:], sel_d[:])
            ones_nb = persist.tile([P, 4 * NB], BF16)
            nc.any.memset(ones_nb[:], 1.0)
            vsum_sb = persist.tile([P, KC], F32)
            nc.sync.dma_start(vsum_sb[:], vsum_d[:])
            cnt32 = persist.tile([NROW, NB], F32)
            nc.sync.dma_start(cnt32[:], cnt_d[:])

            # ---------------- projections ----------------
            with tc.tile_pool(name="proj", bufs=1) as proj:
                qTb = proj.tile([P, KC * S], BF16)
                wq = proj.tile([P, KC * D], BF16)
                wk = proj.tile([P, KC * D], BF16)
                wv = proj.tile([P, KC * D], BF16)
                for kc in range(KC):
                    nc.sync.dma_start(qTb[:, kc * S:(kc + 1) * S],
                                      qT_d[kc * P:(kc + 1) * P, :])
                    nc.sync.dma_start(wq[:, kc * D:(kc + 1) * D], wqT_d[kc * P:(kc + 1) * P, :])
                    nc.sync.dma_start(wk[:, kc * D:(kc + 1) * D], wkT_d[kc * P:(kc + 1) * P, :])
                    nc.sync.dma_start(wv[:, kc * D:(kc + 1) * D], wvT_d[kc * P:(kc + 1) * P, :])
                bq_sb = proj.tile([P, KC], F32)
                bk_sb = proj.tile([P, KC], F32)
                for mc in range(KC):
                    nc.sync.dma_start(bq_sb[:, mc:mc + 1], bq_d[mc * P:(mc + 1) * P, :])
                    nc.sync.dma_start(bk_sb[:, mc:mc + 1], bk_d[mc * P:(mc + 1) * P, :])
                bv_sb = proj.tile([P, D], F32)
                nc.sync.dma_start(bv_sb[:], bv_d[:])

                # Q: per mc one [128,1024] psum; the two 512-col blocks share
                # each (mc,kc) stationary operand (one LDWEIGHTS per pair).
                for mc in range(KC):
                    psf = psS.tile([P, 2 * NB], F32, tag="sps2")
                    for kc in range(KC):
                        w_sl = wq[:, kc * D + mc * P: kc * D + (mc + 1) * P]
                        nc.tensor.matmul(psf[:, 0:NB], w_sl,
                                         qTb[:, kc * S: kc * S + NB],
                                         start=(kc == 0), stop=(kc == KC - 1))
                        nc.tensor.matmul(psf[:, NB:2 * NB], w_sl,
                                         qTb[:, kc * S + NB: kc * S + 2 * NB],
                                         start=(kc == 0), stop=(kc == KC - 1))
                    nc.scalar.activation(qT[:, mc * SH:(mc + 1) * SH], psf[:],
                                         AF.Identity, bias=bq_sb[:, mc:mc + 1])

                # K: per (mc, nb-pair) one [128,1024] psum, paired stationaries
                for mc in range(KC):
                    for nbp in range(2):
                        psf = psS.tile([P, 2 * NB], F32, tag="sps2")
                        for kc in range(KC):
                            w_sl = wk[:, kc * D + mc * P: kc * D + (mc + 1) * P]
                            base = kc * S + nbp * 2 * NB
                            nc.tensor.matmul(psf[:, 0:NB], w_sl,
                                             qTb[:, base: base + NB],
                                             start=(kc == 0), stop=(kc == KC - 1))
                            nc.tensor.matmul(psf[:, NB:2 * NB], w_sl,
                                             qTb[:, base + NB: base + 2 * NB],
                                             start=(kc == 0), stop=(kc == KC - 1))
                        nc.scalar.activation(
                            kT[:, mc * S + nbp * 2 * NB: mc * S + (nbp + 1) * 2 * NB],
                            psf[:], AF.Identity, bias=bk_sb[:, mc:mc + 1])

                # V: per sc one [128,1024] psum (all 16 heads), stationary is
                # the qTb chunk, shared by both 512-col weight blocks.
                for sc in range(SKC):
                    psf = psS.tile([P, 2 * NB], F32, tag="sps2")
                    for kc in range(KC):
                        x_sl = qTb[:, kc * S + sc * P: kc * S + (sc + 1) * P]
                        nc.tensor.matmul(psf[:, 0:NB], x_sl,
                                         wv[:, kc * D: kc * D + NB],
                                         start=(kc == 0), stop=(kc == KC - 1))
                        nc.tensor.matmul(psf[:, NB:2 * NB], x_sl,
                                         wv[:, kc * D + NB: kc * D + 2 * NB],
                                         start=(kc == 0), stop=(kc == KC - 1))
                    vdst3 = vpk[:, sc * VROW:(sc + 1) * VROW].rearrange(
                        "p (h w) -> p h w", h=H)
                    nc.vector.tensor_add(
                        vdst3[:, :, 0:DH],
                        psf[:].rearrange("p (h w) -> p h w", h=H),
                        bv_sb[:].rearrange("p (h w) -> p h w", h=H),
                    )
                    nc.any.memset(vdst3[:, :, DH:VW], -1.0)

            # ---------------- attention ----------------
            with (
                tc.tile_pool(name="mpool", bufs=1) as mpool,
                tc.tile_pool(name="apool", bufs=2) as apool,
                tc.tile_pool(name="work", bufs=2) as work,
            ):
                wo = work.tile([P, KC * D], BF16, tag="wo", bufs=1)
                for kc in range(KC):
                    nc.sync.dma_start(wo[:, kc * D:(kc + 1) * D], woT_d[kc * P:(kc + 1) * P, :])
                bo_sb = work.tile([P, KC], F32, tag="bo", bufs=1)
                for mc in range(KC):
                    nc.sync.dma_start(bo_sb[:, mc:mc + 1], bo_d[mc * P:(mc + 1) * P, :])

                for sq in range(SQB):
                    # chunks 0-7: positive mask M (bf16) for the mul path;
                    # chunks 8-15: inverted mask (uint16) for copy_predicated
                    mTb = mpool.tile([P, 8 * 2 * NB], BF16, tag="mTb")
                    mTu = mpool.tile([P, 8 * 2 * NB], mybir.dt.uint8, tag="mTu", bufs=2)
                    for sc in range(8):
                        for dup in range(2):
                            nc.sync.dma_start(
                                mTb[:, (2 * sc + dup) * NB:(2 * sc + dup + 1) * NB],
                                mT_d[sc * P:(sc + 1) * P, sq * NB:(sq + 1) * NB],
                            )
                    for sc in range(8, SKC):
                        sl = sc - 8
                        for dup in range(2):
                            nc.sync.dma_start(
                                mTu[:, (2 * sl + dup) * NB:(2 * sl + dup + 1) * NB],
                                minvT_d[sc * P:(sc + 1) * P, sq * NB:(sq + 1) * NB],
                            )

                    for pr in range(NPAIR):
                        qs = pr * SH + sq * NB
                        pv0 = psV.tile([P, NB], F32, tag="pv0")
                        pv1 = psV.tile([P, NB], F32, tag="pv1")
                        mc2 = psV.tile([P, NB], F32, tag="mc2")
                        for qt in range(4):  # quarter-passes of 4 sk chunks
                            a01 = apool.tile([P, 4 * 2 * NB], BF16, tag="a01")
                            for sc4 in range(4):
                                sc = qt * 4 + sc4
                                ks = pr * S + sc * P
                                sps = psS.tile([P, 2 * NB], F32, tag="sps2")
                                nc.tensor.matmul(
                                    sps[:, 0:NB], kT[0:DH, ks:ks + P], qT[0:DH, qs:qs + NB],
                                    start=True, stop=True, tile_position=(0, 0),
                                )
                                nc.tensor.matmul(
                                    sps[:, NB:2 * NB], kT[DH:P, ks:ks + P], qT[DH:P, qs:qs + NB],
                                    start=True, stop=True, tile_position=(64, 0),
                                )
                                if sc < 8:
                                    # mask-only correction (depends on mask DMA only)
                                    nc.tensor.matmul(
                                        mc2[0:DH, :],
                                        vpk[:, sc * VROW + (2 * pr) * VW: sc * VROW + (2 * pr) * VW + DH],
                                        mTb[:, 2 * sc * NB: 2 * sc * NB + NB],
                                        start=(sc == 0), stop=(sc == 7),
                                        tile_position=(0, 0),
                                    )
                                    nc.tensor.matmul(
                                        mc2[DH:P, :],
                                        vpk[:, sc * VROW + (2 * pr + 1) * VW: sc * VROW + (2 * pr + 1) * VW + DH],
                                        mTb[:, 2 * sc * NB: 2 * sc * NB + NB],
                                        start=(sc == 0), stop=(sc == 7),
                                        tile_position=(0, 64),
                                    )
                                asl = a01[:, sc4 * 2 * NB:(sc4 + 1) * 2 * NB]
                                nc.scalar.activation(asl, sps[:], AF.Exp, scale=0.125)
                            if qt < 2:
                                # mul path: a01 = E*M, two chunks per op
                                for half in range(2):
                                    lo = half * 2 * 2 * NB
                                    nc.vector.tensor_mul(
                                        a01[:, lo:lo + 2 * 2 * NB],
                                        a01[:, lo:lo + 2 * 2 * NB],
                                        mTb[:, (qt * 4 + half * 2) * 2 * NB:(qt * 4 + half * 2 + 2) * 2 * NB],
                                    )
                            else:
                                # CP path: patch masked slots to 1.0
                                for half in range(2):
                                    lo = half * 2 * 2 * NB
                                    base = ((qt - 2) * 4 + half * 2) * 2 * NB
                                    nc.vector.copy_predicated(
                                        a01[:, lo:lo + 2 * 2 * NB],
                                        mTu[:, base: base + 2 * 2 * NB],
                                        ones_nb[:],
                                    )
                            for h01, pv in ((0, pv0), (1, pv1)):
                                hloc = 2 * pr + h01
                                for sc4 in range(4):
                                    sc = qt * 4 + sc4
                                    nc.tensor.matmul(
                                        pv[0:VW, :],
                                        vpk[:, sc * VROW + hloc * VW: sc * VROW + (hloc + 1) * VW],
                                        a01[:, sc4 * 2 * NB + h01 * NB: sc4 * 2 * NB + (h01 + 1) * NB],
                                        start=(sc == 0), stop=(sc == SKC - 1),
                                    )
                        mcS = work.tile([P, NB], F32, tag="mcS", bufs=2)
                        nc.vector.tensor_scalar(mcS[:], mc2[:],
                                                vsum_sb[:, pr:pr + 1], None,
                                                mybir.AluOpType.add)
                        for h01, pv in ((0, pv0), (1, pv1)):
                            hloc = 2 * pr + h01
                            r = hloc * SQB + sq
                            sstage = work.tile([P, NB], F32, tag="sstage", bufs=3)
                            nc.vector.tensor_copy(sstage[DH:VW, :], pv[DH:VW, :])
                            nc.sync.dma_start(sums_pk[r:r + 1, :], sstage[DH:VW, :])
                            od = pr * SH + sq * NB
                            nc.vector.scalar_tensor_tensor(
                                outMT[h01 * DH:(h01 + 1) * DH, od:od + NB],
                                pv[0:DH, :], -1.0, mcS[h01 * DH:(h01 + 1) * DH, :],
                                mybir.AluOpType.mult, mybir.AluOpType.add,
                            )

                # ---------------- softmax normalization ----------------
                nc.vector.tensor_sub(sums_pk[:], cnt32[:], sums_pk[:])
                nc.vector.reciprocal_approx_fast(recip_pk[:], sums_pk[:])
                nc.vector.tensor_copy(recip_bf[:], recip_pk[:])
                for pr in range(NPAIR):
                    for sq in range(SQB):
                        blk = (pr * SQB + sq) * P
                        bcf = psS.tile([P, 2 * NB], F32, tag="sps2")
                        bc = bcf[:, 0:NB]
                        nc.tensor.matmul(bc[:], sel[:, blk:blk + P],
                                         recip_bf[:], start=True, stop=True)
                        od = pr * SH + sq * NB
                        osl = outMT[:, od:od + NB]
                        nc.vector.tensor_mul(osl, osl, bc[:])

                # ---------------- output projection ----------------
                for mc in range(KC):
                    psf = psS.tile([P, 2 * NB], F32, tag="sps2")
                    for kc in range(KC):
                        w_sl = wo[:, kc * D + mc * P: kc * D + (mc + 1) * P]
                        nc.tensor.matmul(psf[:, 0:NB], w_sl,
                                         outMT[:, kc * SH: kc * SH + NB],
                                         start=(kc == 0), stop=(kc == KC - 1))
                        nc.tensor.matmul(psf[:, NB:2 * NB], w_sl,
                                         outMT[:, kc * SH + NB: kc * SH + 2 * NB],
                                         start=(kc == 0), stop=(kc == KC - 1))
                    fin = work.tile([P, 2 * NB], F32, tag="fin", bufs=3)
                    nc.scalar.activation(fin[:], psf[:], AF.Identity, bias=bo_sb[:, mc:mc + 1])
                    nc.sync.dma_start(out_d[mc * P:(mc + 1) * P, :], fin[:])

    nc.finalize()
    return nc


_NC_CACHE = None
LAST_RESULTS = None


def _get_nc():
    global _NC_CACHE
    if _NC_CACHE is None:
        _NC_CACHE = _build_bass()
    return _NC_CACHE


def kernel(query, mask, Wq, bq, Wk, bk, Wv, bv, Wo, bo, **_unused):
    query = np.asarray(query, dtype=np.float32)
    mask = np.asarray(mask).astype(bool)
    Wq = np.asarray(Wq, dtype=np.float32)
    Wk = np.asarray(Wk, dtype=np.float32)
    Wv = np.asarray(Wv, dtype=np.float32)
    Wo = np.asarray(Wo, dtype=np.float32)
    bq = np.asarray(bq, dtype=np.float32)
    bk = np.asarray(bk, dtype=np.float32)
    bv = np.asarray(bv, dtype=np.float32)
    bo = np.asarray(bo, dtype=np.float32)

    wqT = np.ascontiguousarray(Wq.T).astype(_bf16)
    wkT = np.ascontiguousarray(Wk.T).astype(_bf16)
    wvT = np.ascontiguousarray(-Wv.T).astype(_bf16)   # negated
    woT = np.ascontiguousarray(Wo.T).astype(_bf16)
    bq_c = np.ascontiguousarray(bq.reshape(D, 1))
    bk_c = np.ascontiguousarray(bk.reshape(D, 1))
    bv_x = np.ascontiguousarray(np.broadcast_to(-bv, (P, D)))  # negated
    bo_c = np.ascontiguousarray(bo.reshape(D, 1))
    sel_np = np.zeros((NROW, NPAIR * SQB * P), dtype=np.float32)
    for pr in range(NPAIR):
        for sq in range(SQB):
            blk = (pr * SQB + sq) * P
            sel_np[(2 * pr) * SQB + sq, blk:blk + DH] = 1.0
            sel_np[(2 * pr + 1) * SQB + sq, blk + DH:blk + P] = 1.0
    sel_bf = sel_np.astype(_bf16)

    in_maps = []
    for c in range(NCORES):
        b, half = c // 2, c % 2
        off = half * SH
        qT_rot = np.ascontiguousarray(np.roll(query[b].T, -off, axis=1)).astype(_bf16)
        minv = (~mask[b]).T.astype(np.float32)    # [sk, q], 1.0 where masked
        minv = np.roll(minv, -off, axis=0)
        minvT = np.ascontiguousarray(minv[:, off:off + SH]).astype(np.uint8)
        Msl = 1.0 - minv[:, off:off + SH]          # positive mask, rotated
        mT_c = np.ascontiguousarray(Msl).astype(_bf16)
        # rank-1 corrections restricted to the mul half (rotated sk < 1024)
        cnt_half = (1024.0 - Msl[0:SH, :].sum(axis=0)).astype(np.float32)
        cnt_np = np.zeros((NROW, NB), dtype=np.float32)
        for hloc in range(H):
            for sqq in range(SQB):
                cnt_np[hloc * SQB + sqq, :] = cnt_half[sqq * NB:(sqq + 1) * NB]
        v_full = query[b] @ Wv.T + bv              # [S, D], positive v
        vsum_half = np.roll(v_full, -off, axis=0)[0:SH, :].sum(axis=0)
        vs_np = np.zeros((P, KC), dtype=np.float32)
        for pr in range(NPAIR):
            for p in range(P):
                vs_np[p, pr] = vsum_half[(2 * pr + p // DH) * DH + p % DH]
        in_maps.append({
            "qT": qT_rot,
            "minvT": minvT, "mT": mT_c, "vsum": vs_np, "cnt": cnt_np,
            "wqT": wqT, "wkT": wkT, "wvT": wvT, "woT": woT,
            "bq": bq_c, "bk": bk_c, "bv": bv_x, "bo": bo_c, "sel": sel_bf,
            "out": np.zeros((D, SH), dtype=np.float32),
        })

    nc = _get_nc()
    res = run_bass_kernel_spmd(nc, in_maps, core_ids=list(range(NCORES)))
    global LAST_RESULTS
    LAST_RESULTS = res

    out = np.empty((B, S, D), dtype=np.float32)
    for c in range(NCORES):
        b, half = c // 2, c % 2
        out[b, half * SH:(half + 1) * SH, :] = res.results[c]["out"].T
    return out



# revision 32
# speedup vs baseline: 1.0481x; 1.0481x over previous
"""Multi-head attention (B=4, S=2048, D=1024, H=16) on 8 TRN2 NeuronCores. v3.

Sharding: core c = (batch c//2, seq-half c%2); K/V computed for the full
sequence on every core, Q/attention/outproj only for the core's half; host
rotates the sequence so the core's own half sits at columns [0, SH).
Everything on-chip is transposed (features on partitions): qT/kT [D,s],
scoresT [sk,sq], outT [d,sq].

Softmax: multiplicative mask => masked slots get weight exp(0)=1. ScalarE
exps the raw scores; ONE copy_predicated per 2 sk-chunks (FD=2048, predicate
= host-sent inverted mask, bf16 0/1) patches masked slots to 1.0. Head pairs
share one [128,1024] PSUM scores tile (two K=64 row-tiled matmuls). The
denominator comes free from the PV matmul via a ones column in packed v;
normalization is deferred (reciprocal + one-hot selector broadcast matmul).

v3 vs baseline: copy_predicated at FD=2048 (amortizes the DVE 1x overhead),
projection matmuls paired per stationary operand (the two N=512 column
blocks of each (mc,kc) share one LDWEIGHTS; one FD=1024 bias-activation per
pair), bf16 selector/reciprocal for the broadcast matmul.
"""

import os
import sys

import numpy as np

for _p in ("/opt/trn_rl_repo",):
    if _p not in sys.path and os.path.isdir(_p):
        sys.path.insert(0, _p)

import ml_dtypes

import concourse.bass as bass
from concourse import bacc
import concourse.mybir as mybir
import concourse.tile as tile
from concourse.bass_utils import run_bass_kernel_spmd

BF16 = mybir.dt.bfloat16
F32 = mybir.dt.float32
AF = mybir.ActivationFunctionType

B, S, D, H, DH = 4, 2048, 1024, 16, 64
NCORES = 8
SH = S // 2
P = 128
NB = 512
KC = D // P
SKC = S // P
SQB = SH // NB
NPAIR = H // 2
VW = DH + 1
VROW = H * VW
NROW = H * SQB

_bf16 = ml_dtypes.bfloat16


def _build_bass():
    nc = bacc.Bacc(num_devices=NCORES)

    qT_d = nc.declare_dram_parameter("qT", [D, S], BF16, isOutput=False)
    minvT_d = nc.declare_dram_parameter("minvT", [S, SH], mybir.dt.uint8, isOutput=False)
    mT_d = nc.declare_dram_parameter("mT", [S, SH], BF16, isOutput=False)
    vsum_d = nc.declare_dram_parameter("vsum", [P, KC], F32, isOutput=False)
    cnt_d = nc.declare_dram_parameter("cnt", [NROW, NB], F32, isOutput=False)
    wqT_d = nc.declare_dram_parameter("wqT", [D, D], BF16, isOutput=False)
    wkT_d = nc.declare_dram_parameter("wkT", [D, D], BF16, isOutput=False)
    wvT_d = nc.declare_dram_parameter("wvT", [D, D], BF16, isOutput=False)
    woT_d = nc.declare_dram_parameter("woT", [D, D], BF16, isOutput=False)
    bq_d = nc.declare_dram_parameter("bq", [D, 1], F32, isOutput=False)
    bk_d = nc.declare_dram_parameter("bk", [D, 1], F32, isOutput=False)
    bv_d = nc.declare_dram_parameter("bv", [P, D], F32, isOutput=False)
    bo_d = nc.declare_dram_parameter("bo", [D, 1], F32, isOutput=False)
    sel_d = nc.declare_dram_parameter("sel", [NROW, NPAIR * SQB * P], BF16, isOutput=False)
    out_d = nc.declare_dram_parameter("out", [D, SH], F32, isOutput=True)

    with tile.TileContext(nc) as tc:
        with (
            tc.tile_pool(name="persist", bufs=1) as persist,
            tc.tile_pool(name="psS", bufs=2, space="PSUM") as psS,
            tc.tile_pool(name="psV", bufs=1, space="PSUM") as psV,
        ):
            qT = persist.tile([P, KC * SH], BF16)
            kT = persist.tile([P, KC * S], BF16)
            vpk = persist.tile([P, SKC * VROW], BF16)
            outMT = persist.tile([P, KC * SH], BF16)
            sums_pk = persist.tile([NROW, NB], F32)
            recip_pk = persist.tile([NROW, NB], F32)
            recip_bf = persist.tile([NROW, NB], BF16)
            sel = persist.tile([NROW, NPAIR * SQB * P], BF16)
            nc.sync.dma_start(sel[:], sel_d[:])
            ones_nb = persist.tile([P, 4 * NB], BF16)
            nc.any.memset(ones_nb[:], 1.0)
            vsum_sb = persist.tile([P, KC], F32)
            nc.sync.dma_start(vsum_sb[:], vsum_d[:])
            cnt32 = persist.tile([NROW, NB], F32)
            nc.sync.dma_start(cnt32[:], cnt_d[:])

            # ---------------- projections ----------------
            with tc.tile_pool(name="proj", bufs=1) as proj:
                qTb = proj.tile([P, KC * S], BF16)
                wq = proj.tile([P, KC * D], BF16)
                wk = proj.tile([P, KC * D], BF16)
                wv = proj.tile([P, KC * D], BF16)
                for kc in range(KC):
                    nc.sync.dma_start(qTb[:, kc * S:(kc + 1) * S],
                                      qT_d[kc * P:(kc + 1) * P, :])
                    nc.sync.dma_start(wq[:, kc * D:(kc + 1) * D], wqT_d[kc * P:(kc + 1) * P, :])
                    nc.sync.dma_start(wk[:, kc * D:(kc + 1) * D], wkT_d[kc * P:(kc + 1) * P, :])
                    nc.sync.dma_start(wv[:, kc * D:(kc + 1) * D], wvT_d[kc * P:(kc + 1) * P, :])
                bq_sb = proj.tile([P, KC], F32)
                bk_sb = proj.tile([P, KC], F32)
                for mc in range(KC):
                    nc.sync.dma_start(bq_sb[:, mc:mc + 1], bq_d[mc * P:(mc + 1) * P, :])
                    nc.sync.dma_start(bk_sb[:, mc:mc + 1], bk_d[mc * P:(mc + 1) * P, :])
                bv_sb = proj.tile([P, D], F32)
                nc.sync.dma_start(bv_sb[:], bv_d[:])

                # Q: per mc one [128,1024] psum; the two 512-col blocks share
                # each (mc,kc) stationary operand (one LDWEIGHTS per pair).
                for mc in range(KC):
                    psf = psS.tile([P, 2 * NB], F32, tag="sps2")
                    for kc in range(KC):
                        w_sl = wq[:, kc * D + mc * P: kc * D + (mc + 1) * P]
                        nc.tensor.matmul(psf[:, 0:NB], w_sl,
                                         qTb[:, kc * S: kc * S + NB],
                                         start=(kc == 0), stop=(kc == KC - 1))
                        nc.tensor.matmul(psf[:, NB:2 * NB], w_sl,
                                         qTb[:, kc * S + NB: kc * S + 2 * NB],
                                         start=(kc == 0), stop=(kc == KC - 1))
                    nc.scalar.activation(qT[:, mc * SH:(mc + 1) * SH], psf[:],
                                         AF.Identity, bias=bq_sb[:, mc:mc + 1])

                # K: per (mc, nb-pair) one [128,1024] psum, paired stationaries
                for mc in range(KC):
                    for nbp in range(2):
                        psf = psS.tile([P, 2 * NB], F32, tag="sps2")
                        for kc in range(KC):
                            w_sl = wk[:, kc * D + mc * P: kc * D + (mc + 1) * P]
                            base = kc * S + nbp * 2 * NB
                            nc.tensor.matmul(psf[:, 0:NB], w_sl,
                                             qTb[:, base: base + NB],
                                             start=(kc == 0), stop=(kc == KC - 1))
                            nc.tensor.matmul(psf[:, NB:2 * NB], w_sl,
                                             qTb[:, base + NB: base + 2 * NB],
                                             start=(kc == 0), stop=(kc == KC - 1))
                        nc.scalar.activation(
                            kT[:, mc * S + nbp * 2 * NB: mc * S + (nbp + 1) * 2 * NB],
                            psf[:], AF.Identity, bias=bk_sb[:, mc:mc + 1])

                # V: per sc one [128,1024] psum (all 16 heads), stationary is
                # the qTb chunk, shared by both 512-col weight blocks.
                for sc in range(SKC):
                    psf = psS.tile([P, 2 * NB], F32, tag="sps2")
                    for kc in range(KC):
                        x_sl = qTb[:, kc * S + sc * P: kc * S + (sc + 1) * P]
                        nc.tensor.matmul(psf[:, 0:NB], x_sl,
                                         wv[:, kc * D: kc * D + NB],
                                         start=(kc == 0), stop=(kc == KC - 1))
                        nc.tensor.matmul(psf[:, NB:2 * NB], x_sl,
                                         wv[:, kc * D + NB: kc * D + 2 * NB],
                                         start=(kc == 0), stop=(kc == KC - 1))
                    vdst3 = vpk[:, sc * VROW:(sc + 1) * VROW].rearrange(
                        "p (h w) -> p h w", h=H)
                    nc.vector.tensor_add(
                        vdst3[:, :, 0:DH],
                        psf[:].rearrange("p (h w) -> p h w", h=H),
                        bv_sb[:].rearrange("p (h w) -> p h w", h=H),
                    )
                    nc.any.memset(vdst3[:, :, DH:VW], -1.0)

            # ---------------- attention ----------------
            with (
                tc.tile_pool(name="mpool", bufs=1) as mpool,
                tc.tile_pool(name="apool", bufs=2) as apool,
                tc.tile_pool(name="work", bufs=2) as work,
            ):
                wo = work.tile([P, KC * D], BF16, tag="wo", bufs=1)
                for kc in range(KC):
                    nc.sync.dma_start(wo[:, kc * D:(kc + 1) * D], woT_d[kc * P:(kc + 1) * P, :])
                bo_sb = work.tile([P, KC], F32, tag="bo", bufs=1)
                for mc in range(KC):
                    nc.sync.dma_start(bo_sb[:, mc:mc + 1], bo_d[mc * P:(mc + 1) * P, :])

                for sq in range(SQB):
                    # chunks 0-7: positive mask M (bf16) for the mul path;
                    # chunks 8-15: inverted mask (uint16) for copy_predicated
                    mTb = mpool.tile([P, 8 * 2 * NB], BF16, tag="mTb")
                    mTu = mpool.tile([P, 8 * 2 * NB], mybir.dt.uint8, tag="mTu", bufs=2)
                    for sc in range(8):
                        for dup in range(2):
                            nc.sync.dma_start(
                                mTb[:, (2 * sc + dup) * NB:(2 * sc + dup + 1) * NB],
                                mT_d[sc * P:(sc + 1) * P, sq * NB:(sq + 1) * NB],
                            )
                    for sc in range(8, SKC):
                        sl = sc - 8
                        for dup in range(2):
                            nc.sync.dma_start(
                                mTu[:, (2 * sl + dup) * NB:(2 * sl + dup + 1) * NB],
                                minvT_d[sc * P:(sc + 1) * P, sq * NB:(sq + 1) * NB],
                            )

                    for pr in range(NPAIR):
                        qs = pr * SH + sq * NB
                        pv0 = psV.tile([P, NB], F32, tag="pv0")
                        pv1 = psV.tile([P, NB], F32, tag="pv1")
                        mc2 = psV.tile([P, NB], F32, tag="mc2")

                        def emit_pv(qt, a01, pv0=pv0, pv1=pv1, pr=pr):
                            for h01, pv in ((0, pv0), (1, pv1)):
                                hloc = 2 * pr + h01
                                for sc4 in range(4):
                                    sc = qt * 4 + sc4
                                    nc.tensor.matmul(
                                        pv[0:VW, :],
                                        vpk[:, sc * VROW + hloc * VW: sc * VROW + (hloc + 1) * VW],
                                        a01[:, sc4 * 2 * NB + h01 * NB: sc4 * 2 * NB + (h01 + 1) * NB],
                                        start=(sc == 0), stop=(sc == SKC - 1),
                                    )

                        prev = None
                        for qt in range(4):  # quarter-passes of 4 sk chunks
                            a01 = apool.tile([P, 4 * 2 * NB], BF16, tag="a01")
                            for sc4 in range(4):
                                sc = qt * 4 + sc4
                                ks = pr * S + sc * P
                                sps = psS.tile([P, 2 * NB], F32, tag="sps2")
                                nc.tensor.matmul(
                                    sps[:, 0:NB], kT[0:DH, ks:ks + P], qT[0:DH, qs:qs + NB],
                                    start=True, stop=True, tile_position=(0, 0),
                                )
                                nc.tensor.matmul(
                                    sps[:, NB:2 * NB], kT[DH:P, ks:ks + P], qT[DH:P, qs:qs + NB],
                                    start=True, stop=True, tile_position=(64, 0),
                                )
                                if sc < 8:
                                    # mask-only correction (depends on mask DMA only)
                                    nc.tensor.matmul(
                                        mc2[0:DH, :],
                                        vpk[:, sc * VROW + (2 * pr) * VW: sc * VROW + (2 * pr) * VW + DH],
                                        mTb[:, 2 * sc * NB: 2 * sc * NB + NB],
                                        start=(sc == 0), stop=(sc == 7),
                                        tile_position=(0, 0),
                                    )
                                    nc.tensor.matmul(
                                        mc2[DH:P, :],
                                        vpk[:, sc * VROW + (2 * pr + 1) * VW: sc * VROW + (2 * pr + 1) * VW + DH],
                                        mTb[:, 2 * sc * NB: 2 * sc * NB + NB],
                                        start=(sc == 0), stop=(sc == 7),
                                        tile_position=(0, 64),
                                    )
                                asl = a01[:, sc4 * 2 * NB:(sc4 + 1) * 2 * NB]
                                nc.scalar.activation(asl, sps[:], AF.Exp, scale=0.125)
                            if qt < 2:
                                # mul path: a01 = E*M, two chunks per op
                                for half in range(2):
                                    lo = half * 2 * 2 * NB
                                    nc.vector.tensor_mul(
                                        a01[:, lo:lo + 2 * 2 * NB],
                                        a01[:, lo:lo + 2 * 2 * NB],
                                        mTb[:, (qt * 4 + half * 2) * 2 * NB:(qt * 4 + half * 2 + 2) * 2 * NB],
                                    )
                            else:
                                # CP path: patch masked slots to 1.0
                                for half in range(2):
                                    lo = half * 2 * 2 * NB
                                    base = ((qt - 2) * 4 + half * 2) * 2 * NB
                                    nc.vector.copy_predicated(
                                        a01[:, lo:lo + 2 * 2 * NB],
                                        mTu[:, base: base + 2 * 2 * NB],
                                        ones_nb[:],
                                    )
                            if prev is not None:
                                emit_pv(*prev)
                            prev = (qt, a01)
                        emit_pv(*prev)
                        mcS = work.tile([P, NB], F32, tag="mcS", bufs=2)
                        nc.vector.tensor_scalar(mcS[:], mc2[:],
                                                vsum_sb[:, pr:pr + 1], None,
                                                mybir.AluOpType.add)
                        for h01, pv in ((0, pv0), (1, pv1)):
                            hloc = 2 * pr + h01
                            r = hloc * SQB + sq
                            sstage = work.tile([P, NB], F32, tag="sstage", bufs=3)
                            nc.vector.tensor_copy(sstage[DH:VW, :], pv[DH:VW, :])
                            nc.sync.dma_start(sums_pk[r:r + 1, :], sstage[DH:VW, :])
                            od = pr * SH + sq * NB
                            nc.vector.scalar_tensor_tensor(
                                outMT[h01 * DH:(h01 + 1) * DH, od:od + NB],
                                pv[0:DH, :], -1.0, mcS[h01 * DH:(h01 + 1) * DH, :],
                                mybir.AluOpType.mult, mybir.AluOpType.add,
                            )

                # ---------------- softmax normalization ----------------
                nc.vector.tensor_sub(sums_pk[:], cnt32[:], sums_pk[:])
                nc.vector.reciprocal_approx_fast(recip_pk[:], sums_pk[:])
                nc.vector.tensor_copy(recip_bf[:], recip_pk[:])
                for pr in range(NPAIR):
                    for sq in range(SQB):
                        blk = (pr * SQB + sq) * P
                        bcf = psS.tile([P, 2 * NB], F32, tag="sps2")
                        bc = bcf[:, 0:NB]
                        nc.tensor.matmul(bc[:], sel[:, blk:blk + P],
                                         recip_bf[:], start=True, stop=True)
                        od = pr * SH + sq * NB
                        osl = outMT[:, od:od + NB]
                        nc.vector.tensor_mul(osl, osl, bc[:])

                # ---------------- output projection ----------------
                for mc in range(KC):
                    psf = psS.tile([P, 2 * NB], F32, tag="sps2")
                    for kc in range(KC):
                        w_sl = wo[:, kc * D + mc * P: kc * D + (mc + 1) * P]
                        nc.tensor.matmul(psf[:, 0:NB], w_sl,
                                         outMT[:, kc * SH: kc * SH + NB],
                                         start=(kc == 0), stop=(kc == KC - 1))
                        nc.tensor.matmul(psf[:, NB:2 * NB], w_sl,
                                         outMT[:, kc * SH + NB: kc * SH + 2 * NB],
                                         start=(kc == 0), stop=(kc == KC - 1))
                    fin = work.tile([P, 2 * NB], F32, tag="fin", bufs=3)
                    nc.scalar.activation(fin[:], psf[:], AF.Identity, bias=bo_sb[:, mc:mc + 1])
                    nc.sync.dma_start(out_d[mc * P:(mc + 1) * P, :], fin[:])

    nc.finalize()
    return nc


_NC_CACHE = None
LAST_RESULTS = None


def _get_nc():
    global _NC_CACHE
    if _NC_CACHE is None:
        _NC_CACHE = _build_bass()
    return _NC_CACHE


def kernel(query, mask, Wq, bq, Wk, bk, Wv, bv, Wo, bo, **_unused):
    query = np.asarray(query, dtype=np.float32)
    mask = np.asarray(mask).astype(bool)
    Wq = np.asarray(Wq, dtype=np.float32)
    Wk = np.asarray(Wk, dtype=np.float32)
    Wv = np.asarray(Wv, dtype=np.float32)
    Wo = np.asarray(Wo, dtype=np.float32)
    bq = np.asarray(bq, dtype=np.float32)
    bk = np.asarray(bk, dtype=np.float32)
    bv = np.asarray(bv, dtype=np.float32)
    bo = np.asarray(bo, dtype=np.float32)

    wqT = np.ascontiguousarray(Wq.T).astype(_bf16)
    wkT = np.ascontiguousarray(Wk.T).astype(_bf16)
    wvT = np.ascontiguousarray(-Wv.T).astype(_bf16)   # negated
    woT = np.ascontiguousarray(Wo.T).astype(_bf16)
    bq_c = np.ascontiguousarray(bq.reshape(D, 1))
    bk_c = np.ascontiguousarray(bk.reshape(D, 1))
    bv_x = np.ascontiguousarray(np.broadcast_to(-bv, (P, D)))  # negated
    bo_c = np.ascontiguousarray(bo.reshape(D, 1))
    sel_np = np.zeros((NROW, NPAIR * SQB * P), dtype=np.float32)
    for pr in range(NPAIR):
        for sq in range(SQB):
            blk = (pr * SQB + sq) * P
            sel_np[(2 * pr) * SQB + sq, blk:blk + DH] = 1.0
            sel_np[(2 * pr + 1) * SQB + sq, blk + DH:blk + P] = 1.0
    sel_bf = sel_np.astype(_bf16)

    in_maps = []
    for c in range(NCORES):
        b, half = c // 2, c % 2
        off = half * SH
        qT_rot = np.ascontiguousarray(np.roll(query[b].T, -off, axis=1)).astype(_bf16)
        minv = (~mask[b]).T.astype(np.float32)    # [sk, q], 1.0 where masked
        minv = np.roll(minv, -off, axis=0)
        minvT = np.ascontiguousarray(minv[:, off:off + SH]).astype(np.uint8)
        Msl = 1.0 - minv[:, off:off + SH]          # positive mask, rotated
        mT_c = np.ascontiguousarray(Msl).astype(_bf16)
        # rank-1 corrections restricted to the mul half (rotated sk < 1024)
        cnt_half = (1024.0 - Msl[0:SH, :].sum(axis=0)).astype(np.float32)
        cnt_np = np.zeros((NROW, NB), dtype=np.float32)
        for hloc in range(H):
            for sqq in range(SQB):
                cnt_np[hloc * SQB + sqq, :] = cnt_half[sqq * NB:(sqq + 1) * NB]
        v_full = query[b] @ Wv.T + bv              # [S, D], positive v
        vsum_half = np.roll(v_full, -off, axis=0)[0:SH, :].sum(axis=0)
        vs_np = np.zeros((P, KC), dtype=np.float32)
        for pr in range(NPAIR):
            for p in range(P):
                vs_np[p, pr] = vsum_half[(2 * pr + p // DH) * DH + p % DH]
        in_maps.append({
            "qT": qT_rot,
            "minvT": minvT, "mT": mT_c, "vsum": vs_np, "cnt": cnt_np,
            "wqT": wqT, "wkT": wkT, "wvT": wvT, "woT": woT,
            "bq": bq_c, "bk": bk_c, "bv": bv_x, "bo": bo_c, "sel": sel_bf,
            "out": np.zeros((D, SH), dtype=np.float32),
        })

    nc = _get_nc()
    res = run_bass_kernel_spmd(nc, in_maps, core_ids=list(range(NCORES)))
    global LAST_RESULTS
    LAST_RESULTS = res

    out = np.empty((B, S, D), dtype=np.float32)
    for c in range(NCORES):
        b, half = c // 2, c % 2
        out[b, half * SH:(half + 1) * SH, :] = res.results[c]["out"].T
    return out

